# revision 20
# baseline (speedup 1.0000x reference)
"""Trainium2 Bass kernel for nn_MetaTransformBlock (MoE routing block).

Math (per batch row b):
  h1[k]   = tanh(x_ext[b,k] * w1[k] + b1[k])                [K, D]
  h2[k]   = tanh(h1[k] @ w2[k] + b2[k])                     [K, D]
  W[k]    = LayerNorm(h2[k]) * ln_g + ln_b                  (norm over all D)
  gates   = softmax(tanh(h_prev @ gw1 + gb1) @ gw2 + gb2)   [K]
  theta   = sum_k gates[k] * W[k] + theta0                  [D] -> [8, 64]
  x_prime = x_l @ theta                                     [64]

Sharding: data-parallel over batch across 8 cores (512 rows/core), params
replicated.  Per core / expert k the kernel computes h1 directly in
transposed layout h1T[d, b] (one ACT op per 128-d chunk: tanh(w1*x + b1)
with per-partition scale/bias), then fp32r matmuls accumulate
h1T.T @ w2[k] (+ b2 via a K=1 ones-row matmul) into PSUM [b=128, e=512].
ACT tanh produces h2; DVE bn_stats gives mean/var; the gated LayerNorm
scale g_k/sigma_k is applied via a diagonal matmul accumulating theta in a
persistent PSUM bank (the -g*mu/sigma mean term is accumulated separately
as [128,1] scalars and corrected once at the end, exploiting
sum_k gates = 1 so ln_b/theta0 fold into a single post-add).
"""

import os
import sys

for _p in ("/opt/trn_rl_repo", "/root/.axon_site/_ro/trn_rl_repo"):
    if os.path.isdir(_p) and _p not in sys.path:
        sys.path.insert(0, _p)

from contextlib import ExitStack

import numpy as np

import concourse.bass as bass
import concourse.mybir as mybir
import concourse.tile as tile
from concourse.masks import make_identity

B, H, K = 4096, 256, 16
IN_DIM, XP_DIM = 8, 64
D = IN_DIM * XP_DIM  # 512
NCORES = 8
BL = B // NCORES     # 512 rows per core
NBC = BL // 128      # 4 batch chunks
DC = D // 128        # 4 contraction chunks
EPS = 1e-5

F32 = mybir.dt.float32
BF16 = mybir.dt.bfloat16
AF = mybir.ActivationFunctionType
ALU = mybir.AluOpType
AX = mybir.AxisListType


def _split_multiwaits(nc):
    """The walrus build in this container encodes at most ONE sync-wait per
    instruction ("Too many sync wait commands" otherwise).  Tile's scheduler
    freely emits several; move the extras onto same-engine NOPs inserted
    immediately before, which preserves semantics exactly (the engine just
    blocks a little earlier in its own stream)."""
    ctr = 0
    for f in nc.m.functions:
        for blk in f.blocks:
            new = []
            for inst in blk.instructions:
                si = inst.sync_info
                waits = list(si.on_wait) if si and si.on_wait else []
                if len(waits) > 1:
                    for w in waits[:-1]:
                        ctr += 1
                        nop = mybir.InstEventSemaphore(
                            name=f"waitnop-{ctr}", ins=[], outs=[])
                        nop.engine = inst.engine
                        nop.sync_info = mybir.SyncInfo(on_wait=[w], on_update=[])
                        new.append(nop)
                    inst.sync_info = mybir.SyncInfo(
                        on_wait=[waits[-1]], on_update=list(si.on_update))
                new.append(inst)
            blk.instructions = new
    return ctr


def _row_bcast(row_ap, parts=128):
    """[1, N] DRAM row -> [parts, N] AP with 0 partition stride."""
    inner = [list(p) for p in row_ap.ap[1:]]
    return bass.AP(tensor=row_ap.tensor, offset=row_ap.offset,
                   ap=[[0, parts]] + inner)


def build_nc(debug_taps=False):
    nc = bass.Bass("TRN2", num_devices=NCORES)

    xeT = nc.dram_tensor("xeT", [K, BL], F32, kind="ExternalInput").ap()
    hpT = nc.dram_tensor("hpT", [H, BL], BF16, kind="ExternalInput").ap()
    xl = nc.dram_tensor("xl", [BL, IN_DIM], F32, kind="ExternalInput").ap()
    w1T = nc.dram_tensor("w1T", [D, K], F32, kind="ExternalInput").ap()
    b1T = nc.dram_tensor("b1T", [D, K], F32, kind="ExternalInput").ap()
    w2 = nc.dram_tensor("w2", [K, DC, 128, D], BF16, kind="ExternalInput").ap()
    b2 = nc.dram_tensor("b2", [1, K * D], BF16, kind="ExternalInput").ap()
    gw1 = nc.dram_tensor("gw1", [H, H], BF16, kind="ExternalInput").ap()
    gb1c = nc.dram_tensor("gb1c", [H, 1], F32, kind="ExternalInput").ap()
    gw2 = nc.dram_tensor("gw2", [H, K], BF16, kind="ExternalInput").ap()
    gb2c = nc.dram_tensor("gb2c", [K, 1], F32, kind="ExternalInput").ap()
    lng = nc.dram_tensor("lng", [1, D], F32, kind="ExternalInput").ap()
    lnb = nc.dram_tensor("lnb", [1, D], F32, kind="ExternalInput").ap()
    theta_o = nc.dram_tensor("theta_o", [BL, D], F32, kind="ExternalOutput").ap()
    xp_o = nc.dram_tensor("xp_o", [BL, XP_DIM], F32, kind="ExternalOutput").ap()
    dbg_o = None
    if debug_taps:
        dbg_o = nc.dram_tensor("dbg_o", [128, 256], F32, kind="ExternalOutput").ap()

    with tile.TileContext(nc) as tc:
        with ExitStack() as ctx:
            const = ctx.enter_context(tc.tile_pool(name="const", bufs=1))
            w2p = ctx.enter_context(tc.tile_pool(name="w2p", bufs=2))
            xbp = ctx.enter_context(tc.tile_pool(name="xbp", bufs=2))
            h1p = ctx.enter_context(tc.tile_pool(name="h1p", bufs=2))
            h2p = ctx.enter_context(tc.tile_pool(name="h2p", bufs=3))
            smallp = ctx.enter_context(tc.tile_pool(name="small", bufs=4))
            outp = ctx.enter_context(tc.tile_pool(name="outp", bufs=2))
            psmm = ctx.enter_context(tc.tile_pool(name="psmm", bufs=2, space="PSUM"))
            psth = ctx.enter_context(tc.tile_pool(name="psth", bufs=4, space="PSUM"))

            assert nc.vector.BN_STATS_FMAX >= D

            # ---- constants / params ----
            w1T_sb = const.tile([128, DC, K], F32)
            nc.sync.dma_start(out=w1T_sb, in_=w1T.rearrange("(c p) k -> p c k", p=128))
            b1T_sb = const.tile([128, DC, K], F32)
            nc.sync.dma_start(out=b1T_sb, in_=b1T.rearrange("(c p) k -> p c k", p=128))
            b2_sb = const.tile([1, K, D], BF16)
            nc.sync.dma_start(out=b2_sb, in_=b2)
            gw1_sb = const.tile([128, 2, H], BF16)
            nc.sync.dma_start(out=gw1_sb, in_=gw1.rearrange("(c p) j -> p c j", p=128))
            gb1_sb = const.tile([128, 2, 1], F32)
            nc.sync.dma_start(out=gb1_sb, in_=gb1c.rearrange("(c p) o -> p c o", p=128))
            gw2_sb = const.tile([128, 2, K], BF16)
            nc.sync.dma_start(out=gw2_sb, in_=gw2.rearrange("(c p) k -> p c k", p=128))
            gb2_sb = const.tile([K, 1], F32)
            nc.sync.dma_start(out=gb2_sb, in_=gb2c)
            lng_b = const.tile([128, D], F32)
            nc.sync.dma_start(out=lng_b, in_=_row_bcast(lng))
            lnb_b = const.tile([128, D], F32)
            nc.sync.dma_start(out=lnb_b, in_=_row_bcast(lnb))
            xl_sb = const.tile([128, NBC, IN_DIM], F32)
            nc.sync.dma_start(out=xl_sb, in_=xl.rearrange("(c p) i -> p c i", p=128))
            hpT_sb = const.tile([128, 2, BL], BF16)
            nc.sync.dma_start(out=hpT_sb, in_=hpT.rearrange("(c p) b -> p c b", p=128))

            ident = const.tile([128, 128], BF16)
            make_identity(nc, ident[:])
            ones1 = const.tile([1, 128], BF16)
            nc.vector.memset(ones1[:], 1.0)
            epsT = const.tile([128, 1], F32)
            nc.vector.memset(epsT[:], EPS)

            # ---- gating: gates^T computed in [k, b] layout, then transposed ----
            hidT = const.tile([128, 2, BL], BF16)
            for jc in range(2):
                pmm = psmm.tile([128, 1024], F32, tag="mm")
                ps = pmm[:, :BL]
                for hc in range(2):
                    nc.tensor.matmul(
                        ps,
                        lhsT=gw1_sb[:, hc, jc * 128:(jc + 1) * 128],
                        rhs=hpT_sb[:, hc, :],
                        start=(hc == 0), stop=(hc == 1))
                nc.scalar.activation(out=hidT[:, jc, :], in_=ps, func=AF.Tanh,
                                     bias=gb1_sb[:, jc, :], scale=1.0)

            pmm = psmm.tile([128, 1024], F32, tag="mm")
            psl = pmm[:K, :BL]
            for jc in range(2):
                nc.tensor.matmul(psl, lhsT=gw2_sb[:, jc, :],
                                 rhs=hidT[:, jc, :],
                                 start=(jc == 0), stop=(jc == 1))
            gsrc = const.tile([K, BL], BF16)
            nc.scalar.activation(out=gsrc[:], in_=psl, func=AF.Exp,
                                 bias=gb2_sb[:], scale=1.0)

            dbg = None
            if debug_taps:
                dbg = const.tile([128, 256], F32)
                nc.vector.memset(dbg[:], -7.0)

            gates_sb = const.tile([128, NBC, K], F32)
            for bc in range(NBC):
                pmmt = psmm.tile([128, 2048], BF16, tag="mm")
                pst = pmmt[:, :K]
                nc.tensor.transpose(pst, gsrc[:, bc * 128:(bc + 1) * 128],
                                    ident[:K, :K])
                ssum = smallp.tile([128, 1], F32)
                nc.vector.tensor_reduce(out=ssum[:], in_=pst, axis=AX.X, op=ALU.add)
                rec = smallp.tile([128, 1], F32)
                nc.vector.reciprocal(out=rec[:], in_=ssum[:])
                nc.vector.tensor_scalar(gates_sb[:, bc, :], pst, rec[:], None,
                                        ALU.mult)
                if debug_taps and bc == 0:
                    nc.vector.tensor_copy(out=dbg[:, 64:64 + K], in_=pst)
                    nc.vector.tensor_copy(out=dbg[:, 80:81], in_=ssum[:])
                    nc.vector.tensor_copy(out=dbg[:, 81:82], in_=rec[:])
            if debug_taps:
                nc.vector.tensor_copy(
                    out=dbg[:, 0:NBC * K],
                    in_=gates_sb[:].rearrange("p a b -> p (a b)"))

            # ---- main loop over experts ----
            th_ps = []
            for bc in range(NBC):
                tp = psth.tile([128, D], F32, tag="theta")
                th_ps.append(tp)
            amu = const.tile([128, NBC, K], F32)

            for k in range(K):
                w2_sb = w2p.tile([128, DC, D], BF16)
                for c in range(DC):
                    nc.sync.dma_start(out=w2_sb[:, c, :], in_=w2[k, c])
                xb = xbp.tile([128, BL], F32)
                nc.sync.dma_start(out=xb, in_=_row_bcast(xeT[k:k + 1, :]))

                h1T = h1p.tile([128, DC, BL], BF16)
                for c in range(DC):
                    nc.scalar.activation(out=h1T[:, c, :], in_=xb[:], func=AF.Tanh,
                                         bias=b1T_sb[:, c, k:k + 1],
                                         scale=w1T_sb[:, c, k:k + 1])
                if debug_taps and k == 0:
                    nc.vector.tensor_copy(out=dbg[:, 192:256], in_=h1T[:, 0, 0:64])
                    nc.vector.tensor_copy(out=dbg[:, 102:103], in_=xb[:, 0:1])

                for pr in range(NBC // 2):
                    pmm = psmm.tile([128, 1024], F32, tag="mm")
                    for half in range(2):
                        bc = pr * 2 + half
                        ps = pmm[:, half * D:(half + 1) * D]
                        for c in range(DC):
                            nc.tensor.matmul(
                                ps,
                                lhsT=h1T[:, c, bc * 128:(bc + 1) * 128],
                                rhs=w2_sb[:, c, :],
                                start=(c == 0), stop=False)
                        nc.tensor.matmul(ps, lhsT=ones1[:],
                                         rhs=b2_sb[:, k, :],
                                         start=False, stop=True)
                    h2 = h2p.tile([128, 1024], BF16)
                    nc.scalar.activation(out=h2[:], in_=pmm[:], func=AF.Tanh)
                    for half in range(2):
                        bc = pr * 2 + half
                        h2h = h2[:, half * D:(half + 1) * D]
                        st6 = smallp.tile([128, 6], F32)
                        nc.vector.bn_stats(out=st6[:], in_=h2h)
                        mv = smallp.tile([128, 2], F32)
                        nc.vector.bn_aggr(out=mv[:], in_=st6[:])
                        sd = smallp.tile([128, 1], F32)
                        nc.scalar.activation(out=sd[:], in_=mv[:, 1:2], func=AF.Sqrt,
                                             bias=epsT[:], scale=1.0)
                        inv = smallp.tile([128, 1], F32)
                        nc.vector.reciprocal(out=inv[:], in_=sd[:])
                        a = smallp.tile([128, 1], F32)
                        nc.vector.tensor_scalar(a[:], inv[:],
                                                gates_sb[:, bc, k:k + 1], None,
                                                ALU.mult)
                        nc.vector.tensor_scalar(amu[:, bc, k:k + 1], mv[:, 0:1],
                                                a[:], None, ALU.mult)
                        if debug_taps and k == 0 and bc == 0:
                            nc.vector.tensor_copy(out=dbg[:, 96:98], in_=mv[:])
                            nc.vector.tensor_copy(out=dbg[:, 98:99], in_=sd[:])
                            nc.vector.tensor_copy(out=dbg[:, 99:100], in_=inv[:])
                            nc.vector.tensor_copy(out=dbg[:, 100:101], in_=a[:])
                            nc.vector.tensor_copy(out=dbg[:, 128:192],
                                                  in_=h2h[:, 0:64])
                        dg = smallp.tile([128, 128], BF16)
                        nc.vector.tensor_scalar(dg[:], ident[:], a[:], None, ALU.mult)
                        nc.tensor.matmul(th_ps[bc][:], lhsT=dg[:],
                                         rhs=h2h,
                                         start=(k == 0), stop=(k == K - 1),
                                         skip_group_check=True)

            # ---- epilogue: mean-correction, ln affine, outputs ----
            for bc in range(NBC):
                cac = smallp.tile([128, 1], F32)
                nc.vector.tensor_reduce(out=cac[:], in_=amu[:, bc, :],
                                        axis=AX.X, op=ALU.add)
                tmp = outp.tile([128, D], F32, tag="tmp")
                nc.vector.tensor_scalar(tmp[:], th_ps[bc][:], cac[:], None,
                                        ALU.subtract)
                nc.vector.tensor_tensor(tmp[:], tmp[:], lng_b[:], ALU.mult)
                tht = outp.tile([128, D], F32, tag="tht")
                nc.vector.tensor_tensor(tht[:], tmp[:], lnb_b[:], ALU.add)
                nc.sync.dma_start(out=theta_o[bc * 128:(bc + 1) * 128, :], in_=tht[:])

                prod = outp.tile([128, XP_DIM, IN_DIM], F32, tag="prod")
                thv = tht[:].rearrange("p (i e) -> p i e", i=IN_DIM).transpose([0, 2, 1])
                xlv = xl_sb[:, bc, :].unsqueeze(1).broadcast_to([128, XP_DIM, IN_DIM])
                nc.vector.tensor_tensor(prod[:], thv, xlv, ALU.mult)
                xp = outp.tile([128, XP_DIM], F32, tag="xp")
                nc.vector.tensor_reduce(out=xp[:], in_=prod[:], axis=AX.X, op=ALU.add)
                nc.sync.dma_start(out=xp_o[bc * 128:(bc + 1) * 128, :], in_=xp[:])

            if debug_taps:
                nc.sync.dma_start(out=dbg_o, in_=dbg[:])

    _split_multiwaits(nc)
    return nc


_NC = None


def _get_nc():
    global _NC
    if _NC is None:
        _NC = build_nc()
    return _NC


def make_in_maps(inputs):
    import ml_dtypes
    BFNP = ml_dtypes.bfloat16
    f = lambda x: np.ascontiguousarray(np.asarray(x), dtype=np.float32)
    fb = lambda x: np.ascontiguousarray(np.asarray(x, dtype=np.float32).astype(BFNP))
    h_prevT = f(inputs["h_prev_rnn"]).T
    xeT = f(inputs["x_ext"]).T
    x_l = f(inputs["x_l"])
    shared = dict(
        w1T=f(f(inputs["meta_w1"]).T),
        b1T=f(f(inputs["meta_b1"]).T),
        w2=fb(f(inputs["meta_w2"]).reshape(K, DC, 128, D)),
        b2=fb(f(inputs["meta_b2"]).reshape(1, K * D)),
        gw1=fb(inputs["gw1"]),
        gb1c=f(inputs["gb1"]).reshape(H, 1),
        gw2=fb(inputs["gw2"]),
        gb2c=f(inputs["gb2"]).reshape(K, 1),
        lng=f(inputs["ln_g"]).reshape(1, D),
        lnb=f(f(inputs["ln_b"]) + f(inputs["theta0"])[0]).reshape(1, D),
    )
    in_maps = []
    for c in range(NCORES):
        sl = slice(c * BL, (c + 1) * BL)
        m = dict(shared)
        m["xeT"] = np.ascontiguousarray(xeT[:, sl])
        m["hpT"] = np.ascontiguousarray(h_prevT[:, sl].astype(BFNP))
        m["xl"] = np.ascontiguousarray(x_l[sl])
        in_maps.append(m)
    return in_maps


def assemble_outputs(results):
    theta = np.concatenate([r["theta_o"] for r in results], axis=0)
    theta = theta.reshape(B, IN_DIM, XP_DIM)
    x_prime = np.concatenate([r["xp_o"] for r in results], axis=0)
    return x_prime, theta


def run(inputs, trace=False, **kw):
    from concourse.bass_utils import run_bass_kernel_spmd
    nc = _get_nc()
    in_maps = make_in_maps(inputs)
    res = run_bass_kernel_spmd(nc, in_maps, list(range(NCORES)), trace=trace, **kw)
    x_prime, theta = assemble_outputs(res.results)
    return (x_prime, theta), res


def kernel(**inputs):
    out, _ = run(inputs, trace=False)
    return out


# revision 23
# speedup vs baseline: 1.6306x; 1.6306x over previous
"""Trainium2 Bass kernel for nn_MetaTransformBlock (MoE routing block).

Math (per batch row b):
  h1[k]   = tanh(x_ext[b,k] * w1[k] + b1[k])                [K, D]
  h2[k]   = tanh(h1[k] @ w2[k] + b2[k])                     [K, D]
  W[k]    = LayerNorm(h2[k]) * ln_g + ln_b                  (norm over all D)
  gates   = softmax(tanh(h_prev @ gw1 + gb1) @ gw2 + gb2)   [K]
  theta   = sum_k gates[k] * W[k] + theta0                  [D] -> [8, 64]
  x_prime = x_l @ theta                                     [64]

Sharding: data-parallel over batch across 8 cores (512 rows/core), params
replicated.  Per core / expert k the kernel computes h1 directly in
transposed layout h1T[d, b] (one ACT op per 128-d chunk: tanh(w1*x + b1)
with per-partition scale/bias), then fp32r matmuls accumulate
h1T.T @ w2[k] (+ b2 via a K=1 ones-row matmul) into PSUM [b=128, e=512].
ACT tanh produces h2; DVE bn_stats gives mean/var; the gated LayerNorm
scale g_k/sigma_k is applied via a diagonal matmul accumulating theta in a
persistent PSUM bank (the -g*mu/sigma mean term is accumulated separately
as [128,1] scalars and corrected once at the end, exploiting
sum_k gates = 1 so ln_b/theta0 fold into a single post-add).
"""

import os
import sys

for _p in ("/opt/trn_rl_repo", "/root/.axon_site/_ro/trn_rl_repo"):
    if os.path.isdir(_p) and _p not in sys.path:
        sys.path.insert(0, _p)

from contextlib import ExitStack

import numpy as np

import concourse.bass as bass
import concourse.mybir as mybir
import concourse.tile as tile
from concourse.masks import make_identity

B, H, K = 4096, 256, 16
IN_DIM, XP_DIM = 8, 64
D = IN_DIM * XP_DIM  # 512
NCORES = 8
BL = B // NCORES     # 512 rows per core
NBC = BL // 128      # 4 batch chunks
DC = D // 128        # 4 contraction chunks
EPS = 1e-5

F32 = mybir.dt.float32
BF16 = mybir.dt.bfloat16
AF = mybir.ActivationFunctionType
ALU = mybir.AluOpType
AX = mybir.AxisListType
I32 = mybir.dt.int32


def _split_multiwaits(nc):
    """The walrus build in this container encodes at most ONE sync-wait per
    instruction ("Too many sync wait commands" otherwise).  Tile's scheduler
    freely emits several; move the extras onto same-engine NOPs inserted
    immediately before, which preserves semantics exactly (the engine just
    blocks a little earlier in its own stream)."""
    ctr = 0
    for f in nc.m.functions:
        for blk in f.blocks:
            new = []
            for inst in blk.instructions:
                si = inst.sync_info
                waits = list(si.on_wait) if si and si.on_wait else []
                if len(waits) > 1:
                    for w in waits[:-1]:
                        ctr += 1
                        nop = mybir.InstEventSemaphore(
                            name=f"waitnop-{ctr}", ins=[], outs=[])
                        nop.engine = inst.engine
                        nop.sync_info = mybir.SyncInfo(on_wait=[w], on_update=[])
                        new.append(nop)
                    inst.sync_info = mybir.SyncInfo(
                        on_wait=[waits[-1]], on_update=list(si.on_update))
                new.append(inst)
            blk.instructions = new
    return ctr


def _row_bcast(row_ap, parts=128):
    """[1, N] DRAM row -> [parts, N] AP with 0 partition stride."""
    inner = [list(p) for p in row_ap.ap[1:]]
    return bass.AP(tensor=row_ap.tensor, offset=row_ap.offset,
                   ap=[[0, parts]] + inner)


def build_nc(debug_taps=False, split_waits=True):
    nc = bass.Bass("TRN2", num_devices=NCORES)

    xeT = nc.dram_tensor("xeT", [K, BL], F32, kind="ExternalInput").ap()
    hpT = nc.dram_tensor("hpT", [H, BL], BF16, kind="ExternalInput").ap()
    xl = nc.dram_tensor("xl", [BL, IN_DIM], F32, kind="ExternalInput").ap()
    w1T = nc.dram_tensor("w1T", [D, K], F32, kind="ExternalInput").ap()
    b1T = nc.dram_tensor("b1T", [D, K], F32, kind="ExternalInput").ap()
    w2 = nc.dram_tensor("w2", [K, DC, 128, D], BF16, kind="ExternalInput").ap()
    b2 = nc.dram_tensor("b2", [1, K * D], BF16, kind="ExternalInput").ap()
    gw1 = nc.dram_tensor("gw1", [H, H], BF16, kind="ExternalInput").ap()
    gb1c = nc.dram_tensor("gb1c", [H, 1], F32, kind="ExternalInput").ap()
    gw2 = nc.dram_tensor("gw2", [H, K], BF16, kind="ExternalInput").ap()
    gb2c = nc.dram_tensor("gb2c", [K, 1], F32, kind="ExternalInput").ap()
    lng = nc.dram_tensor("lng", [1, D], F32, kind="ExternalInput").ap()
    lnb = nc.dram_tensor("lnb", [1, D], F32, kind="ExternalInput").ap()
    theta_o = nc.dram_tensor("theta_o", [BL, D], F32, kind="ExternalOutput").ap()
    xp_o = nc.dram_tensor("xp_o", [BL, XP_DIM], F32, kind="ExternalOutput").ap()
    dbg_o = None
    if debug_taps:
        dbg_o = nc.dram_tensor("dbg_o", [128, 256], F32, kind="ExternalOutput").ap()

    with tile.TileContext(nc) as tc:
        with ExitStack() as ctx:
            const = ctx.enter_context(tc.tile_pool(name="const", bufs=1))
            w2p = ctx.enter_context(tc.tile_pool(name="w2p", bufs=2))
            xbp = ctx.enter_context(tc.tile_pool(name="xbp", bufs=2))
            h1p = ctx.enter_context(tc.tile_pool(name="h1p", bufs=2))
            h2p = ctx.enter_context(tc.tile_pool(name="h2p", bufs=3))
            smallp = ctx.enter_context(tc.tile_pool(name="small", bufs=4))
            outp = ctx.enter_context(tc.tile_pool(name="outp", bufs=2))
            psmm = ctx.enter_context(tc.tile_pool(name="psmm", bufs=2, space="PSUM"))
            psth = ctx.enter_context(tc.tile_pool(name="psth", bufs=4, space="PSUM"))

            assert nc.vector.BN_STATS_FMAX >= D

            # ---- constants / params ----
            w1T_sb = const.tile([128, DC, K], F32)
            nc.sync.dma_start(out=w1T_sb, in_=w1T.rearrange("(c p) k -> p c k", p=128))
            b1T_sb = const.tile([128, DC, K], F32)
            nc.sync.dma_start(out=b1T_sb, in_=b1T.rearrange("(c p) k -> p c k", p=128))
            b2_sb = const.tile([1, K, D], BF16)
            nc.sync.dma_start(out=b2_sb, in_=b2)
            gw1_sb = const.tile([128, 2, H], BF16)
            nc.sync.dma_start(out=gw1_sb, in_=gw1.rearrange("(c p) j -> p c j", p=128))
            gb1_sb = const.tile([128, 2, 1], F32)
            nc.sync.dma_start(out=gb1_sb, in_=gb1c.rearrange("(c p) o -> p c o", p=128))
            gw2_sb = const.tile([128, 2, K], BF16)
            nc.sync.dma_start(out=gw2_sb, in_=gw2.rearrange("(c p) k -> p c k", p=128))
            gb2_sb = const.tile([K, 1], F32)
            nc.sync.dma_start(out=gb2_sb, in_=gb2c)
            lng_b = const.tile([128, D], F32)
            nc.sync.dma_start(out=lng_b, in_=_row_bcast(lng))
            lnb_b = const.tile([128, D], F32)
            nc.sync.dma_start(out=lnb_b, in_=_row_bcast(lnb))
            xl_sb = const.tile([128, NBC, IN_DIM], F32)
            nc.sync.dma_start(out=xl_sb, in_=xl.rearrange("(c p) i -> p c i", p=128))
            hpT_sb = const.tile([128, 2, BL], BF16)
            nc.sync.dma_start(out=hpT_sb, in_=hpT.rearrange("(c p) b -> p c b", p=128))

            ident = const.tile([128, 128], BF16)
            make_identity(nc, ident[:])
            ones1 = const.tile([1, 128], BF16)
            nc.vector.memset(ones1[:], 1.0)
            epsT = const.tile([128, 1], F32)
            nc.vector.memset(epsT[:], EPS)

            # ---- gating: gates^T computed in [k, b] layout, then transposed ----
            hidT = const.tile([128, 2, BL], BF16)
            for jc in range(2):
                pmm = psmm.tile([128, 1024], F32, tag="mm")
                ps = pmm[:, :BL]
                for hc in range(2):
                    nc.tensor.matmul(
                        ps,
                        lhsT=gw1_sb[:, hc, jc * 128:(jc + 1) * 128],
                        rhs=hpT_sb[:, hc, :],
                        start=(hc == 0), stop=(hc == 1))
                nc.scalar.activation(out=hidT[:, jc, :], in_=ps, func=AF.Tanh,
                                     bias=gb1_sb[:, jc, :], scale=1.0)

            pmm = psmm.tile([128, 1024], F32, tag="mm")
            psl = pmm[:K, :BL]
            for jc in range(2):
                nc.tensor.matmul(psl, lhsT=gw2_sb[:, jc, :],
                                 rhs=hidT[:, jc, :],
                                 start=(jc == 0), stop=(jc == 1))
            gsrc = const.tile([K, BL], BF16)
            nc.scalar.activation(out=gsrc[:], in_=psl, func=AF.Exp,
                                 bias=gb2_sb[:], scale=1.0)

            dbg = None
            if debug_taps:
                dbg = const.tile([128, 256], F32)
                nc.vector.memset(dbg[:], -7.0)

            gates_sb = const.tile([128, NBC, K], F32)
            for bc in range(NBC):
                pmmt = psmm.tile([128, 2048], BF16, tag="mm")
                pst = pmmt[:, :K]
                nc.tensor.transpose(pst, gsrc[:, bc * 128:(bc + 1) * 128],
                                    ident[:K, :K])
                ssum = smallp.tile([128, 1], F32)
                nc.vector.tensor_reduce(out=ssum[:], in_=pst, axis=AX.X, op=ALU.add)
                rec = smallp.tile([128, 1], F32)
                nc.vector.reciprocal(out=rec[:], in_=ssum[:])
                nc.vector.tensor_scalar(gates_sb[:, bc, :], pst, rec[:], None,
                                        ALU.mult)
                if debug_taps and bc == 0:
                    nc.vector.tensor_copy(out=dbg[:, 64:64 + K], in_=pst)
                    nc.vector.tensor_copy(out=dbg[:, 80:81], in_=ssum[:])
                    nc.vector.tensor_copy(out=dbg[:, 81:82], in_=rec[:])
            if debug_taps:
                nc.vector.tensor_copy(
                    out=dbg[:, 0:NBC * K],
                    in_=gates_sb[:].rearrange("p a b -> p (a b)"))

            # ---- main loop over experts ----
            th_ps = []
            for bc in range(NBC):
                tp = psth.tile([128, D], F32, tag="theta")
                th_ps.append(tp)
            amu = const.tile([128, NBC, K], F32)

            for k in range(K):
                w2_sb = w2p.tile([128, DC, D], BF16)
                for c in range(DC):
                    nc.sync.dma_start(out=w2_sb[:, c, :], in_=w2[k, c])
                xb = xbp.tile([128, BL], F32)
                nc.sync.dma_start(out=xb, in_=_row_bcast(xeT[k:k + 1, :]))

                h2s = []
                svar = smallp.tile([128, NBC, 2], F32)
                h1T = h1p.tile([128, DC, BL], BF16)
                for c in range(DC):
                    nc.scalar.activation(out=h1T[:, c, :], in_=xb[:], func=AF.Tanh,
                                         bias=b1T_sb[:, c, k:k + 1],
                                         scale=w1T_sb[:, c, k:k + 1])
                if debug_taps and k == 0:
                    nc.vector.tensor_copy(out=dbg[:, 192:256], in_=h1T[:, 0, 0:64])
                    nc.vector.tensor_copy(out=dbg[:, 102:103], in_=xb[:, 0:1])

                for pr in range(NBC // 2):
                    pmm = psmm.tile([128, 1024], F32, tag="mm")
                    for half in range(2):
                        bc = pr * 2 + half
                        ps = pmm[:, half * D:(half + 1) * D]
                        for c in range(DC):
                            nc.tensor.matmul(
                                ps,
                                lhsT=h1T[:, c, bc * 128:(bc + 1) * 128],
                                rhs=w2_sb[:, c, :],
                                start=(c == 0), stop=False)
                        nc.tensor.matmul(ps, lhsT=ones1[:],
                                         rhs=b2_sb[:, k, :],
                                         start=False, stop=True)
                    h2 = h2p.tile([128, 1024], BF16, tag="h2" + str(pr))
                    nc.scalar.activation(out=h2[:], in_=pmm[:], func=AF.Tanh)
                    h2s.append(h2)
                    for half in range(2):
                        bc = pr * 2 + half
                        st6 = smallp.tile([128, 6], F32)
                        nc.vector.bn_stats(out=st6[:],
                                           in_=h2[:, half * D:(half + 1) * D])
                        nc.vector.bn_aggr(out=svar[:, bc, :], in_=st6[:])
                # rsqrt(var+eps) for all 4 b-chunks on DVE.  No ACT Sqrt: a
                # Tanh<->Sqrt table reload costs ~1.3us each way.  Quake III
                # initial guess + 2 Newton steps; feeds a bf16 diag matmul so
                # the precision is ample.
                xv = smallp.tile([128, NBC], F32)
                nc.vector.tensor_scalar(xv[:], svar[:, :, 1], EPS, None, ALU.add)
                yb = smallp.tile([128, NBC], F32)
                nc.vector.tensor_scalar(yb[:].bitcast(I32),
                                        xv[:].bitcast(I32),
                                        1, None, ALU.arith_shift_right)
                nc.vector.tensor_scalar(yb[:].bitcast(I32),
                                        yb[:].bitcast(I32),
                                        -1, 0x5f3759df, ALU.mult, ALU.add)
                xh = smallp.tile([128, NBC], F32)
                nc.vector.tensor_scalar(xh[:], xv[:], -0.5, None, ALU.mult)
                for _ in range(2):
                    t1 = smallp.tile([128, NBC], F32)
                    nc.vector.tensor_tensor(t1[:], yb[:], yb[:], ALU.mult)
                    nc.vector.tensor_tensor(t1[:], t1[:], xh[:], ALU.mult)
                    nc.vector.tensor_scalar(t1[:], t1[:], 1.5, None, ALU.add)
                    nc.vector.tensor_tensor(yb[:], yb[:], t1[:], ALU.mult)
                a4 = smallp.tile([128, NBC], F32)
                nc.vector.tensor_tensor(a4[:], yb[:], gates_sb[:, :, k], ALU.mult)
                nc.vector.tensor_tensor(amu[:, :, k], svar[:, :, 0], a4[:], ALU.mult)
                if debug_taps and k == 0:
                    nc.vector.tensor_copy(out=dbg[:, 96:98], in_=svar[:, 0, :])
                    nc.vector.tensor_copy(out=dbg[:, 99:100], in_=yb[:, 0:1])
                    nc.vector.tensor_copy(out=dbg[:, 100:101], in_=a4[:, 0:1])
                    nc.vector.tensor_copy(out=dbg[:, 128:192], in_=h2s[0][:, 0:64])
                for bc in range(NBC):
                    h2h = h2s[bc // 2][:, (bc % 2) * D:(bc % 2 + 1) * D]
                    dg = smallp.tile([128, 128], BF16)
                    nc.vector.tensor_scalar(dg[:], ident[:], a4[:, bc:bc + 1],
                                            None, ALU.mult)
                    nc.tensor.matmul(th_ps[bc][:], lhsT=dg[:],
                                     rhs=h2h,
                                     start=(k == 0), stop=(k == K - 1),
                                     skip_group_check=True)

            # ---- epilogue: mean-correction, ln affine, outputs ----
            for bc in range(NBC):
                cac = smallp.tile([128, 1], F32)
                nc.vector.tensor_reduce(out=cac[:], in_=amu[:, bc, :],
                                        axis=AX.X, op=ALU.add)
                tmp = outp.tile([128, D], F32, tag="tmp")
                nc.vector.tensor_scalar(tmp[:], th_ps[bc][:], cac[:], None,
                                        ALU.subtract)
                nc.vector.tensor_tensor(tmp[:], tmp[:], lng_b[:], ALU.mult)
                tht = outp.tile([128, D], F32, tag="tht")
                nc.vector.tensor_tensor(tht[:], tmp[:], lnb_b[:], ALU.add)
                nc.sync.dma_start(out=theta_o[bc * 128:(bc + 1) * 128, :], in_=tht[:])

                prod = outp.tile([128, XP_DIM, IN_DIM], F32, tag="prod")
                thv = tht[:].rearrange("p (i e) -> p i e", i=IN_DIM).transpose([0, 2, 1])
                xlv = xl_sb[:, bc, :].unsqueeze(1).broadcast_to([128, XP_DIM, IN_DIM])
                nc.vector.tensor_tensor(prod[:], thv, xlv, ALU.mult)
                xp = outp.tile([128, XP_DIM], F32, tag="xp")
                nc.vector.tensor_reduce(out=xp[:], in_=prod[:], axis=AX.X, op=ALU.add)
                nc.sync.dma_start(out=xp_o[bc * 128:(bc + 1) * 128, :], in_=xp[:])

            if debug_taps:
                nc.sync.dma_start(out=dbg_o, in_=dbg[:])

    if split_waits:
        _split_multiwaits(nc)
    return nc


_NC = None


def _get_nc():
    global _NC
    if _NC is None:
        _NC = build_nc()
    return _NC


def make_in_maps(inputs):
    import ml_dtypes
    BFNP = ml_dtypes.bfloat16
    f = lambda x: np.ascontiguousarray(np.asarray(x), dtype=np.float32)
    fb = lambda x: np.ascontiguousarray(np.asarray(x, dtype=np.float32).astype(BFNP))
    h_prevT = f(inputs["h_prev_rnn"]).T
    xeT = f(inputs["x_ext"]).T
    x_l = f(inputs["x_l"])
    shared = dict(
        w1T=f(f(inputs["meta_w1"]).T),
        b1T=f(f(inputs["meta_b1"]).T),
        w2=fb(f(inputs["meta_w2"]).reshape(K, DC, 128, D)),
        b2=fb(f(inputs["meta_b2"]).reshape(1, K * D)),
        gw1=fb(inputs["gw1"]),
        gb1c=f(inputs["gb1"]).reshape(H, 1),
        gw2=fb(inputs["gw2"]),
        gb2c=f(inputs["gb2"]).reshape(K, 1),
        lng=f(inputs["ln_g"]).reshape(1, D),
        lnb=f(f(inputs["ln_b"]) + f(inputs["theta0"])[0]).reshape(1, D),
    )
    in_maps = []
    for c in range(NCORES):
        sl = slice(c * BL, (c + 1) * BL)
        m = dict(shared)
        m["xeT"] = np.ascontiguousarray(xeT[:, sl])
        m["hpT"] = np.ascontiguousarray(h_prevT[:, sl].astype(BFNP))
        m["xl"] = np.ascontiguousarray(x_l[sl])
        in_maps.append(m)
    return in_maps


def assemble_outputs(results):
    theta = np.concatenate([r["theta_o"] for r in results], axis=0)
    theta = theta.reshape(B, IN_DIM, XP_DIM)
    x_prime = np.concatenate([r["xp_o"] for r in results], axis=0)
    return x_prime, theta


def run(inputs, trace=False, **kw):
    from concourse.bass_utils import run_bass_kernel_spmd
    nc = _get_nc()
    in_maps = make_in_maps(inputs)
    res = run_bass_kernel_spmd(nc, in_maps, list(range(NCORES)), trace=trace, **kw)
    x_prime, theta = assemble_outputs(res.results)
    return (x_prime, theta), res


def kernel(**inputs):
    out, _ = run(inputs, trace=False)
    return out
